# revision 1
# baseline (speedup 1.0000x reference)
"""Trainium2 Bass kernel for AxialSelfAttention2d (see reference in module docstring).

Reference computation (per batch b):
    qkv = W @ x + b            (1x1 conv; W [3E, E], x [E, S, L], E = 512)
    q, k, v split; q *= Dh**-0.5; per head h: q,k,v [Dh=64, S, L]
    col:  scores[s,t|l] = q[:,s,l].k[:,t,l]; softmax over t; out_col = attn @ v
    row:  scores[l,m|s] = q[:,s,l].k[:,s,m]; softmax over m; out_row = attn @ v
    out = out_col + out_row    -> [H*Dh, S, L]

Sharding: 8 cores = 2 batches x 4 head-pairs. Each core computes 2 heads of one
batch end-to-end (no collectives); the host concatenates core outputs.

Per-core dataflow (matmul operands fp16, fp32 PSUM accumulation):
  A)  x fp32 --cast-DMA--> SBUF fp16 tiles; QKV projection with W^T stationary
      -> q2, k2, v_sl [128(2h x 64d), S*L] fp16 (+ bias, q pre-scaled on host).
  A2) v_ls = v_sl reordered to (l,s) free order (gpsimd copy);
      vT_row[h][l, s*65+{d,1}] <- DMA-transpose(v_sl[h]);
      vT_col[h][s, l*65+{d,1}] <- DMA-transpose(v_ls[h]); ones columns memset.
  B)  col attention per (l, h): scoresT[t,s] = k_l^T @ q_l (PE, K=64, two heads
      row-packed via base partitions); e = exp(scoresT) (ACT, no max-subtraction
      -- scores are ~N(0,1)); AV: out[s, 65] = e^T.T @ vT_col_l (column 64 gives
      the softmax denominator); fused DVE divide (denominator broadcast with a
      step-0 free dim) -> col_src[s, l*128+hd].
  B2) DMA-transpose col_src chunks -> dst[hd, s*128+l] (final orientation).
  C)  row attention symmetric -> row_src[l, s*128+hd]; DMA-transpose chunks;
      DVE add into dst; cast-DMA (fp16 -> fp32) to DRAM out.
"""

import numpy as np
from contextlib import ExitStack

NUM_HEADS = 8
DIM_HEAD = 64
EMBED = 512
B, S, L = 2, 128, 128
SL = S * L
N_CORES = 8
HPC = 2  # heads per core

_CACHE = {}


def build_program(nc, tc):
    import concourse.bass as bass
    import concourse.mybir as mybir

    f16 = mybir.dt.float16
    f32 = mybir.dt.float32
    AF = mybir.ActivationFunctionType
    OP = mybir.AluOpType
    AP = bass.AP

    x_d = nc.dram_tensor("x", [EMBED, S, L], f32, kind="ExternalInput")
    w_d = nc.dram_tensor("wT", [EMBED, 384], f16, kind="ExternalInput")
    b_d = nc.dram_tensor("bvec", [384], f32, kind="ExternalInput")
    out_d = nc.dram_tensor("out", [128, S, L], f32, kind="ExternalOutput")

    x_flat = x_d.ap().rearrange("c s l -> c (s l)")

    CH = 32          # slice indices per chunk
    NCH = 128 // CH  # 4

    def stage_a(qk_pool, q2, k2, v_sl):
        GW = 2048  # spatial columns per x load
        with tc.tile_pool(name="xload", bufs=2) as xpool, \
             tc.tile_pool(name="wpool", bufs=1) as wpool, \
             tc.tile_pool(name="qkvps", bufs=4, space="PSUM") as qkv_ps:
            w_sb = wpool.tile([128, 4, 384], f16, tag="w")
            nc.sync.dma_start(w_sb[:],
                              w_d.ap().rearrange("(k c) o -> c k o", k=4))
            b_sb = wpool.tile([128, 3], f32, tag="b")
            nc.sync.dma_start(b_sb[:], b_d.ap().rearrange("(m p) -> p m", p=128))
            for g in range(SL // GW):
                xt = xpool.tile([128, 4, GW], f16, tag="x")
                nc.gpsimd.dma_start(
                    xt[:],
                    x_flat[:, g * GW:(g + 1) * GW]
                        .rearrange("(k c) n -> c k n", k=4))
                for m in range(3):  # 0=q, 1=k, 2=v
                    dest = (q2, k2, v_sl)[m]
                    for sg in range(GW // 512):
                        ps = qkv_ps.tile([128, 512], f32, tag="acc")
                        for c in range(4):
                            nc.tensor.matmul(
                                ps[:],
                                w_sb[:][:, c, m * 128:(m + 1) * 128],
                                xt[:][:, c, sg * 512:(sg + 1) * 512],
                                start=(c == 0), stop=(c == 3))
                        off = g * GW + sg * 512
                        nc.vector.tensor_scalar_add(
                            dest[:][:, off:off + 512], ps[:],
                            b_sb[:][:, m:m + 1])

    def make_vt(pool, tmp_pool, tagp, src, n_outer):
        """vt[h][p, i*65 + {0..63: d, 64: 1}] <- transpose of src[h-slice].

        DMA-transpose requires a packed [p, mid, last] output (strided mid
        corrupts data on HW), so transpose into a packed tmp then let gpsimd
        restride into the 65-wide augmented layout."""
        vts = []
        for h in range(HPC):
            vt = pool.tile([128, n_outer * 65], f16, tag=f"{tagp}{h}")
            for qtr in range(n_outer // 32):
                tmp = tmp_pool.tile([128, 32, 64], f16, tag="vtmp")
                nc.sync.dma_start(
                    tmp[:],
                    src[:][h * 64:(h + 1) * 64,
                           qtr * 32 * 128:(qtr + 1) * 32 * 128],
                    transpose=True)
                o = AP(vt[:].tensor, vt[:].offset + qtr * 32 * 65,
                       [list(vt[:].ap[0]), [65, 32], [1, 64]])
                nc.gpsimd.tensor_copy(o, tmp[:])
            ones_ap = AP(vt[:].tensor, vt[:].offset + 64,
                         [list(vt[:].ap[0]), [65, n_outer], [1, 1]])
            nc.vector.memset(ones_ap, 1.0)
            vts.append(vt)
        return vts

    # ---------------- attention (direction 0 = col, 1 = row) ----------------
    def attention(direction, vt, qv, kv, dst, zero_sb):
        with ExitStack() as dctx:
            src_pool = dctx.enter_context(
                tc.tile_pool(name=f"src{direction}", bufs=2))
            sc_ps = dctx.enter_context(
                tc.tile_pool(name=f"scps{direction}", bufs=2, space="PSUM"))
            av_ps = dctx.enter_context(
                tc.tile_pool(name=f"avps{direction}", bufs=2, space="PSUM"))
            e_pool = dctx.enter_context(
                tc.tile_pool(name=f"e{direction}", bufs=4))
            den_pool = dctx.enter_context(
                tc.tile_pool(name=f"den{direction}", bufs=2))
            tr_pool = None
            if direction == 0:
                tr_pool = dctx.enter_context(tc.tile_pool(name="coltr", bufs=2))

            if direction == 0:
                def qk_slice(t, h, i):  # [64, t/s] column i, stride L
                    return t[h * 64:(h + 1) * 64, :, i]
            else:
                def qk_slice(t, h, i):  # [64, m/l] row i, contiguous
                    return t[h * 64:(h + 1) * 64, i, :]

            # PSUM-bank discipline: matmuls with different tile_positions must
            # never write the same bank (HW fault) -> per-head score banks.
            for ch in range(NCH):
                src = src_pool.tile([128, CH * 128], f16, tag="src")
                for quad in range(CH // 4):
                    i0 = ch * CH + quad * 4
                    for h in range(2):
                        sc = sc_ps.tile([128, 512], f32, tag=f"sc{h}")
                        for j in range(4):
                            nc.tensor.matmul(
                                sc[:][:, j * 128:(j + 1) * 128],
                                qk_slice(kv, h, i0 + j),
                                qk_slice(qv, h, i0 + j),
                                start=True, stop=True)
                        et = e_pool.tile([128, 512], f16, tag="e")
                        nc.scalar.activation(et[:], sc[:], AF.Exp,
                                             bias=zero_sb[:][:, 0:1])
                        av = av_ps.tile([128, 260], f32, tag="av")
                        for j in range(4):
                            nc.tensor.matmul(
                                av[:][:, j * 65:(j + 1) * 65],
                                et[:][:, j * 128:(j + 1) * 128],
                                vt[h][:][:, (i0 + j) * 65:(i0 + j + 1) * 65],
                                start=True, stop=True)
                        den = den_pool.tile([128, 4], f32, tag="den")
                        nc.vector.reciprocal(
                            den[:], AP(av[:].tensor, av[:].offset + 64,
                                       [list(av[:].ap[0]), [65, 4]]))
                        # src[s, (i0+j)*128 + h*64 + d] = av[:, j*65+d]*rden[:, j]
                        in0 = AP(av[:].tensor, av[:].offset,
                                 [list(av[:].ap[0]), [65, 4], [1, 64]])
                        in1 = AP(den[:].tensor, den[:].offset,
                                 [list(den[:].ap[0]), [1, 4], [0, 64]])
                        o = AP(src[:].tensor,
                               src[:].offset + (quad * 4) * 128 + h * 64,
                               [list(src[:].ap[0]), [128, 4], [1, 64]])
                        nc.vector.tensor_tensor(o, in0, in1, OP.mult)

                if direction == 1:
                    # row runs first: transpose lands directly in dst
                    # dst[hd, (ch*CH+sr)*128 + l'] <- src[l', sr*128+hd]
                    od = AP(dst[:].tensor, dst[:].offset + ch * CH * 128,
                            [list(dst[:].ap[0]), [128, CH], [1, 128]])
                    nc.sync.dma_start(od, src[:], transpose=True)
                else:
                    # col: transpose to trc[hd, lr*128 + s], then strided add
                    tr = tr_pool.tile([128, CH * 128], f16, tag="tr")
                    ot = AP(tr[:].tensor, tr[:].offset,
                            [list(tr[:].ap[0]), [128, CH], [1, 128]])
                    nc.sync.dma_start(ot, src[:], transpose=True)
                    # dst[hd, s*128 + (ch*CH+lr)] += trc[hd, lr*128 + s]
                    dseg = AP(dst[:].tensor, dst[:].offset + ch * CH,
                              [list(dst[:].ap[0]), [1, CH], [128, S]])
                    nc.vector.tensor_add(dseg, dseg, tr[:])

    # ---------------- top-level pool nesting (LIFO) ----------------
    import os
    stage = os.environ.get("AXIAL_DEBUG_STAGE", "full")
    with tc.tile_pool(name="qk", bufs=1) as qk_pool, \
         tc.tile_pool(name="vt", bufs=1) as vt_pool:
        q2 = qk_pool.tile([128, SL], f16, tag="q2")
        k2 = qk_pool.tile([128, SL], f16, tag="k2")
        zero_sb = qk_pool.tile([128, 1], f32, tag="z")
        nc.vector.memset(zero_sb[:], 0.0)

        with tc.tile_pool(name="vsl", bufs=1) as vsl_pool:
            v_sl = vsl_pool.tile([128, SL], f16, tag="v_sl")
            stage_a(qk_pool, q2, k2, v_sl)
            vt_row = vt_col = None
            if stage != "a":
                with tc.tile_pool(name="vtmp", bufs=2) as tmp_pool:
                    vt_row = make_vt(vt_pool, tmp_pool, "vtr", v_sl, S)
                    with tc.tile_pool(name="vls", bufs=1) as vls_pool:
                        v_ls = vls_pool.tile([128, SL], f16, tag="v_ls")
                        nc.gpsimd.tensor_copy(
                            v_ls[:].rearrange("p (l s) -> p l s", s=S),
                            v_sl[:].rearrange("p (s l) -> p l s", l=L))
                        vt_col = make_vt(vt_pool, tmp_pool, "vtc", v_ls, L)

        with tc.tile_pool(name="dstp", bufs=1) as dst_pool:
            dst = dst_pool.tile([128, SL], f16, tag="dst")  # [hd, s*128+l]
            qv = q2[:].rearrange("p (s l) -> p s l", l=L)
            kv = k2[:].rearrange("p (s l) -> p s l", l=L)
            if stage in ("row", "full"):
                attention(1, vt_row, qv, kv, dst, zero_sb)  # row: fills dst
            if stage == "full":
                attention(0, vt_col, qv, kv, dst, zero_sb)  # col: adds
            if stage in ("a", "a2"):
                nc.vector.tensor_copy(dst[:], q2[:])
            for ch in range(NCH):
                nc.gpsimd.dma_start(
                    out_d.ap()[:, ch * CH:(ch + 1) * CH, :],
                    dst[:][:, ch * CH * 128:(ch + 1) * CH * 128]
                        .rearrange("p (s l) -> p s l", l=L))


def _get_nc():
    if "nc" in _CACHE:
        return _CACHE["nc"]
    import concourse.bacc as bacc
    import concourse.tile as tile

    nc = bacc.Bacc(None, target_bir_lowering=False, debug=False,
                   num_devices=N_CORES)
    with tile.TileContext(nc) as tc:
        build_program(nc, tc)
    nc.compile()
    _CACHE["nc"] = nc
    return nc


def make_in_maps(x, W, b):
    x = np.asarray(x, dtype=np.float32)
    W = np.asarray(W, dtype=np.float32)
    b = np.asarray(b, dtype=np.float32)
    scale = np.float32(DIM_HEAD ** -0.5)
    in_maps = []
    for c in range(N_CORES):
        bb, h0 = c // 4, 2 * (c % 4)
        hd = np.arange(h0 * 64, (h0 + 2) * 64)
        sel = np.concatenate([hd, EMBED + hd, 2 * EMBED + hd])
        W_loc = W[sel, :].copy()
        b_loc = b[sel].copy()
        W_loc[:128] *= scale
        b_loc[:128] *= scale
        in_maps.append({
            "x": np.ascontiguousarray(x[bb]),
            "wT": np.ascontiguousarray(W_loc.T).astype(np.float16),
            "bvec": b_loc.astype(np.float32),
        })
    return in_maps


def assemble(results):
    out = np.empty((B, EMBED, S, L), dtype=np.float32)
    for c, r in enumerate(results):
        bb, h0 = c // 4, 2 * (c % 4)
        out[bb, h0 * 64:(h0 + 2) * 64] = r["out"]
    return out


def kernel(x, W, b):
    from concourse.bass_utils import run_bass_kernel_spmd
    nc = _get_nc()
    res = run_bass_kernel_spmd(nc, make_in_maps(x, W, b),
                               core_ids=list(range(N_CORES)))
    return assemble(res.results)



# revision 27
# speedup vs baseline: 6.3412x; 6.3412x over previous
"""Trainium2 Bass kernel for AxialSelfAttention2d.

Reference computation (per batch b):
    qkv = W @ x + b            (1x1 conv; W [3E, E], x [E, S, L], E = 512)
    q, k, v split; q *= Dh**-0.5; per head h: q,k,v [Dh=64, S, L]
    col:  scores[s,t|l] = q[:,s,l].k[:,t,l]; softmax over t; out_col = attn @ v
    row:  scores[l,m|s] = q[:,s,l].k[:,s,m]; softmax over m; out_row = attn @ v
    out = out_col + out_row    -> [H*Dh, S, L]

Sharding: 8 cores = 2 batches x 4 head-pairs. Each core computes 2 heads of one
batch end-to-end (no collectives); the host concatenates core outputs.

Per-core dataflow (fp16 I/O, fp16 matmul operands, fp32 PSUM):
  A)  x fp16 (host-cast) -> SBUF via SWDGE; QKV projection with W^T stationary
      -> q2, k2, v_sl [128(2h x 64d), S*L] fp16 (+ bias via alternating
      DVE / ACT PSUM evacuation; q pre-scaled on host).
  A2) v_ls = v_sl reordered to (l,s) (gpsimd); vt tiles [128, 128, 64] per head
      <- direct DMA-transpose chunks (packed, no restride). vt tiles are built
      for the row direction first, then overwritten for col after row's AVs.
  B)  row attention per (s, h): scoresT[m, l] = k_s^T q_s (PE, K=64, two heads
      via base partitions); e = exp (ACT); AV: av[l, j*64+d] = e^T.T @ vt_j
      (N=64) plus denominator column av[, 256+j] via N=1 matmul against a ones
      vector (same stationary); DVE reciprocal + fused divide-multiply ->
      src[l, s*128+hd]; DMA-transpose chunks directly into dst[hd, s*128+l].
  C)  col attention symmetric -> src[s, l*128+hd]; DMA-transpose chunks to
      tr[hd, l*128+s]; gpsimd strided add into dst; fp16 DMA out per chunk.
"""

import numpy as np
from contextlib import ExitStack

NUM_HEADS = 8
DIM_HEAD = 64
EMBED = 512
B, S, L = 2, 128, 128
SL = S * L
N_CORES = 8
HPC = 2  # heads per core

_CACHE = {}


def build_program(nc, tc):
    import concourse.bass as bass
    import concourse.mybir as mybir

    f16 = mybir.dt.float16
    f32 = mybir.dt.float32
    AF = mybir.ActivationFunctionType
    OP = mybir.AluOpType
    AP = bass.AP

    x_d = nc.dram_tensor("x", [EMBED, S, L], f16, kind="ExternalInput")
    w_d = nc.dram_tensor("wT", [EMBED, 384], f16, kind="ExternalInput")
    b_d = nc.dram_tensor("bvec", [384], f32, kind="ExternalInput")
    out_d = nc.dram_tensor("out", [128, S, L], f16, kind="ExternalOutput")

    x_flat = x_d.ap().rearrange("c s l -> c (s l)")

    CH = 32          # slice indices per chunk
    NCH = 128 // CH  # 4
    GW = 1024        # spatial columns per x load

    # ramp-up chunking: small first loads so PE starts early
    CHUNKS = [512, 512] + [1024] * 15
    assert sum(CHUNKS) == SL

    def stage_a(q2, k2, v_sl, b_sb, w_sb):
        with tc.tile_pool(name="xload", bufs=3) as xpool, \
             tc.tile_pool(name="qkvps", bufs=4, space="PSUM") as qkv_ps:
            evac_i = 0
            off0 = 0
            for gw in CHUNKS:
                xt = xpool.tile([128, 4, GW], f16, tag="x")
                nc.gpsimd.dma_start(
                    xt[:][:, :, 0:gw],
                    x_flat[:, off0:off0 + gw]
                        .rearrange("(k c) n -> c k n", k=4))
                for m in range(3):  # 0=q, 1=k, 2=v
                    dest = (q2, k2, v_sl)[m]
                    sgo = 0
                    while sgo < gw:
                        pw = min(1024, gw - sgo)  # PSUM tile width
                        ps = qkv_ps.tile([128, 1024], f32, tag="acc")
                        for sg2 in range(pw // 512):
                            for c in range(4):
                                nc.tensor.matmul(
                                    ps[:][:, sg2 * 512:(sg2 + 1) * 512],
                                    w_sb[:][:, c, m * 128:(m + 1) * 128],
                                    xt[:][:, c, sgo + sg2 * 512:
                                          sgo + (sg2 + 1) * 512],
                                    start=(c == 0), stop=(c == 3))
                        off = off0 + sgo
                        # alternate PSUM evacuation between DVE and ACT
                        if evac_i % 2 == 0:
                            nc.vector.tensor_scalar_add(
                                dest[:][:, off:off + pw], ps[:][:, 0:pw],
                                b_sb[:][:, m:m + 1])
                        else:
                            nc.scalar.activation(
                                dest[:][:, off:off + pw], ps[:][:, 0:pw],
                                AF.Identity, bias=b_sb[:][:, m:m + 1])
                        evac_i += 1
                        sgo += pw
                off0 += gw

    def make_vt(vts, src):
        """vts[h] [128, 128, 64]: vts[h][p, i, d] <- src[h*64+d, i*128 + p]."""
        for h in range(HPC):
            for qtr in range(4):
                nc.sync.dma_start(
                    vts[h][:][:, qtr * 32:(qtr + 1) * 32, :],
                    src[:][h * 64:(h + 1) * 64,
                           qtr * 32 * 128:(qtr + 1) * 32 * 128],
                    transpose=True)

    # ---------------- attention (direction 0 = col, 1 = row) ----------------
    def attention(direction, pools, vt, qv, kv, dst, zero_sb, ones_sb,
                  out_fn=None, post_chunk_fn=None):
        src_pool, sc_ps, av_ps, den_ps, e_pool, den_pool, tr_pool = pools
        if True:
            if direction == 0:
                def qk_slice(t, h, i):  # [64, t/s] column i, stride L
                    return t[h * 64:(h + 1) * 64, :, i]
            else:
                def qk_slice(t, h, i):  # [64, m/l] row i, contiguous
                    return t[h * 64:(h + 1) * 64, i, :]

            # Uneven chunks: a small final chunk shortens the endgame (its
            # transpose + adds + out DMAs are all that remain after the last
            # matmul).
            CHS = (48, 48, 24, 8)
            ch_start = 0
            for ci, chw in enumerate(CHS):
                src = src_pool.tile([128, max(CHS) * 128], f16, tag="src")
                tr = None
                if direction == 0:
                    if ci == len(CHS) - 1:
                        tr = tr_pool.tile([128, CHS[-1] * 128], f16, tag="trs")
                    else:
                        tr = tr_pool.tile([128, max(CHS) * 128], f16, tag="tr")
                for qp in range(chw // 8):
                    i0 = ch_start + qp * 8
                    for h in range(2):
                        sc = sc_ps.tile([128, 1024], f32, tag=f"sc{h}")
                        for j in range(8):
                            nc.tensor.matmul(
                                sc[:][:, j * 128:(j + 1) * 128],
                                qk_slice(kv, h, i0 + j),
                                qk_slice(qv, h, i0 + j),
                                start=True, stop=True)
                        et = e_pool.tile([128, 1024], f16, tag="e")
                        nc.scalar.activation(et[:], sc[:], AF.Exp,
                                             bias=zero_sb[:][:, 0:1])
                        # av: 8x64 AV blocks (one bank); dens in their own bank
                        av = av_ps.tile([128, 512], f32, tag="av")
                        dn = den_ps.tile([128, 8], f32, tag="dn")
                        for j in range(8):
                            nc.tensor.matmul(
                                av[:][:, j * 64:(j + 1) * 64],
                                et[:][:, j * 128:(j + 1) * 128],
                                vt[h][:][:, i0 + j, :],
                                start=True, stop=True)
                            nc.tensor.matmul(
                                dn[:][:, j:j + 1],
                                et[:][:, j * 128:(j + 1) * 128],
                                ones_sb[:][:, 0:1],
                                start=True, stop=True)
                        den = den_pool.tile([128, 8], f32, tag="den")
                        nc.vector.reciprocal(den[:], dn[:])
                        # src[p, (i0+j)*128 + h*64 + d] = av[:, j*64+d]*rden[:, j]
                        in0 = AP(av[:].tensor, av[:].offset,
                                 [list(av[:].ap[0]), [64, 8], [1, 64]])
                        in1 = AP(den[:].tensor, den[:].offset,
                                 [list(den[:].ap[0]), [1, 8], [0, 64]])
                        o = AP(src[:].tensor,
                               src[:].offset + (qp * 8) * 128 + h * 64,
                               [list(src[:].ap[0]), [128, 8], [1, 64]])
                        nc.vector.tensor_tensor(o, in0, in1, OP.mult)

                    # per-qp-pair transpose right after both heads' divides,
                    # spreading DMA work instead of bunching it per chunk
                    sslc = src[:][:, qp * 1024:(qp + 1) * 1024]
                    if direction == 1:
                        # row runs first: transpose lands directly in dst
                        # dst[hd, (i0+sr)*128 + l'] <- src[l', (qp*8+sr)*128+hd]
                        od = AP(dst[:].tensor, dst[:].offset + i0 * 128,
                                [list(dst[:].ap[0]), [128, 8], [1, 128]])
                        nc.sync.dma_start(od, sslc, transpose=True)
                    else:
                        # col: transpose to trc[hd, lr*128 + s]
                        ot = AP(tr[:].tensor, tr[:].offset + qp * 1024,
                                [list(tr[:].ap[0]), [128, 8], [1, 128]])
                        nc.sync.dma_start(ot, sslc, transpose=True)

                if direction == 0:
                    # dst[hd, s*128+(ch_start+lr)] += trc[hd, lr*128+s], split
                    # per s-chunk so the out DMA for s-chunk c2 fires right
                    # after the last column-chunk's sub-add for it
                    for c2 in range(NCH):
                        dseg = AP(dst[:].tensor,
                                  dst[:].offset + ch_start + c2 * CH * 128,
                                  [list(dst[:].ap[0]), [1, chw], [128, CH]])
                        trs = AP(tr[:].tensor, tr[:].offset + c2 * CH,
                                 [list(tr[:].ap[0]), [128, chw], [1, CH]])
                        eng = nc.gpsimd if ci < 1 else nc.vector
                        eng.tensor_tensor(dseg, dseg, trs, OP.add)
                        if ci == len(CHS) - 1:
                            out_fn(c2)
                if post_chunk_fn is not None:
                    post_chunk_fn(ch_start + chw)
                ch_start += chw

    # ---------------- top-level pool nesting (LIFO) ----------------
    with tc.tile_pool(name="qk", bufs=1) as qk_pool, \
         tc.tile_pool(name="vt", bufs=1) as vt_pool, \
         tc.tile_pool(name="vls", bufs=1) as vls_pool:
        q2 = qk_pool.tile([128, SL], f16, tag="q2")
        k2 = qk_pool.tile([128, SL], f16, tag="k2")
        zero_sb = qk_pool.tile([128, 1], f32, tag="z")
        nc.vector.memset(zero_sb[:], 0.0)
        ones_sb = qk_pool.tile([128, 1], f16, tag="ones")
        nc.vector.memset(ones_sb[:], 1.0)
        w_sb = qk_pool.tile([128, 4, 384], f16, tag="w")
        nc.sync.dma_start(w_sb[:],
                          w_d.ap().rearrange("(k c) o -> c k o", k=4))
        b_sb = qk_pool.tile([128, 3], f32, tag="b")
        nc.sync.dma_start(b_sb[:], b_d.ap().rearrange("(m p) -> p m", p=128))

        vts = []
        for h in range(HPC):
            vth = vt_pool.tile([128, 128, 64], f16, tag=f"vt{h}")
            vts.append(vth)
        v_ls = vls_pool.tile([128, SL], f16, tag="v_ls")

        with tc.tile_pool(name="vsl", bufs=1) as vsl_pool:
            v_sl = vsl_pool.tile([128, SL], f16, tag="v_sl")
            stage_a(q2, k2, v_sl, b_sb, w_sb)
            # v_ls split per l-quarter so each vt_col transpose can start as
            # soon as its quarter is reordered; spread over DVE + gpsimd so
            # all quarters land within ~2 copies' time
            for qtr in range(4):
                o = AP(v_ls[:].tensor, v_ls[:].offset + qtr * 32 * 128,
                       [list(v_ls[:].ap[0]), [128, 32], [1, 128]])
                i = AP(v_sl[:].tensor, v_sl[:].offset + qtr * 32,
                       [list(v_sl[:].ap[0]), [1, 32], [128, 128]])
                eng = nc.vector if qtr < 2 else nc.gpsimd
                eng.tensor_copy(o, i)
            make_vt(vts, v_sl)  # row orientation

        with tc.tile_pool(name="dstp", bufs=1) as dst_pool:
            dst = dst_pool.tile([128, SL], f16, tag="dst")  # [hd, s*128+l]
            qv = q2[:].rearrange("p (s l) -> p s l", l=L)
            kv = k2[:].rearrange("p (s l) -> p s l", l=L)
            def out_fn(c2):
                nc.sync.dma_start(
                    out_d.ap()[:, c2 * CH:(c2 + 1) * CH, :],
                    dst[:][:, c2 * CH * 128:(c2 + 1) * CH * 128]
                        .rearrange("p (s l) -> p s l", l=L))

            with ExitStack() as dctx:
                # shared across both directions so no pool-close barrier
                # separates them; PSUM: sc 2 tags x 1 x 2 banks + av 2 x 2 = 8
                pools = (
                    dctx.enter_context(tc.tile_pool(name="src", bufs=2)),
                    dctx.enter_context(
                        tc.tile_pool(name="scps", bufs=1, space="PSUM")),
                    dctx.enter_context(
                        tc.tile_pool(name="avps", bufs=3, space="PSUM")),
                    dctx.enter_context(
                        tc.tile_pool(name="denps", bufs=1, space="PSUM")),
                    dctx.enter_context(tc.tile_pool(name="epool", bufs=3)),
                    dctx.enter_context(tc.tile_pool(name="denp", bufs=4)),
                    dctx.enter_context(tc.tile_pool(name="coltr", bufs=1)),
                )
                done_qtr = [0]

                def rebuild_vt_col(ch_end):
                    # after a row chunk covering i < ch_end, vt quarters below
                    # ch_end//32 are no longer read by row AVs -> overwrite
                    # them with the col orientation right away
                    while done_qtr[0] < ch_end // 32:
                        qtr = done_qtr[0]
                        for h in range(HPC):
                            nc.sync.dma_start(
                                vts[h][:][:, qtr * 32:(qtr + 1) * 32, :],
                                v_ls[:][h * 64:(h + 1) * 64,
                                        qtr * 32 * 128:(qtr + 1) * 32 * 128],
                                transpose=True)
                        done_qtr[0] += 1

                attention(1, pools, vts, qv, kv, dst, zero_sb, ones_sb,
                          post_chunk_fn=rebuild_vt_col)
                attention(0, pools, vts, qv, kv, dst, zero_sb, ones_sb,
                          out_fn=out_fn)  # col: adds + pipelined out DMA


def _get_nc():
    if "nc" in _CACHE:
        return _CACHE["nc"]
    import concourse.bacc as bacc
    import concourse.tile as tile

    nc = bacc.Bacc(None, target_bir_lowering=False, debug=False,
                   num_devices=N_CORES)
    with tile.TileContext(nc) as tc:
        build_program(nc, tc)
    nc.compile()
    _CACHE["nc"] = nc
    return nc


def make_in_maps(x, W, b):
    x = np.asarray(x, dtype=np.float32)
    W = np.asarray(W, dtype=np.float32)
    b = np.asarray(b, dtype=np.float32)
    scale = np.float32(DIM_HEAD ** -0.5)
    in_maps = []
    for c in range(N_CORES):
        bb, h0 = c // 4, 2 * (c % 4)
        hd = np.arange(h0 * 64, (h0 + 2) * 64)
        sel = np.concatenate([hd, EMBED + hd, 2 * EMBED + hd])
        W_loc = W[sel, :].copy()
        b_loc = b[sel].copy()
        W_loc[:128] *= scale
        b_loc[:128] *= scale
        in_maps.append({
            "x": np.ascontiguousarray(x[bb]).astype(np.float16),
            "wT": np.ascontiguousarray(W_loc.T).astype(np.float16),
            "bvec": b_loc.astype(np.float32),
        })
    return in_maps


def assemble(results):
    out = np.empty((B, EMBED, S, L), dtype=np.float32)
    for c, r in enumerate(results):
        bb, h0 = c // 4, 2 * (c % 4)
        out[bb, h0 * 64:(h0 + 2) * 64] = r["out"].astype(np.float32)
    return out


def kernel(x, W, b):
    from concourse.bass_utils import run_bass_kernel_spmd
    nc = _get_nc()
    res = run_bass_kernel_spmd(nc, make_in_maps(x, W, b),
                               core_ids=list(range(N_CORES)))
    return assemble(res.results)
